# revision 7
# baseline (speedup 1.0000x reference)
"""GroupViT AssignAttention (eval branch) on 8 Trainium2 NeuronCores.

Data-parallel over batch: 32 batches -> 4 per core. Each core runs an
identical Bass/Tile program on its own batch shard; no collectives.

Math (per batch, G=64 groups, N=4096 tokens, C=384):
  q = query @ q_w.T + q_b ; k = key @ k_w.T + k_b ; v = key @ v_w.T + v_b
  raw = (q @ k.T) * s,  s = C**-0.5
  soft = softmax_G(raw)                                    [output 2]
  y_hard = onehot(argmax_G(soft))  (forward value of the STE expression)
  attn = y_hard / (y_hard.sum_N + 1)
  out = (attn @ v) @ p_w.T + p_b                           [output 1]

Key algebraic restructuring (exact up to fp rounding):
  raw[g,n] = query@ (q_w.T @ k_w) @ key.T + c1[g] + c2[n] + c3
  Per-token constants c2, c3 shift every logit of a softmax-over-G column
  equally -> dropped. A = q_w.T@k_w and u1 = q_w.T@k_b are precomputed on
  the host from weights only; c1 = query @ u1 on device. This removes the
  full K projection from the device entirely.
  The V path aggregates one-hot-selected v rows via PE matmul with an
  appended ones column producing per-group counts; v bias and p bias are
  folded in afterwards via a rank-1 matmul (cnt x (p_w@v_b)) and a fused
  scalar_tensor_tensor epilogue.

Layouts: everything feature-major ([C,*], C split into 3 chunks of 128
partitions). Attention logits are produced directly transposed,
attnT [n-tokens(partitions), 64 groups(free)], so softmax / argmax /
one-hot are free-dim vector ops and Y@V contracts over token partitions.

Precision: argmax-critical path (tT, attnT) in fp32 PE matmuls; V path
and output projection in fp32r (reduced-mantissa fp32, full PE rate,
~1.5e-4 relative error - far below thresholds for these smooth outputs).
"""

import numpy as np

import concourse.bass as bass
import concourse.bacc as bacc
import concourse.mybir as mybir
import concourse.tile as tile
from concourse import bass_utils
from concourse.masks import make_identity

F32 = mybir.dt.float32
F32R = mybir.dt.float32r

B, G, N, C = 32, 64, 4096, 384
NCORES = 8
BPC = B // NCORES  # batches per core
KC = C // 128  # feature chunks (3)
NCH = N // 128  # token chunks per batch (32)
GRP = 8  # token chunks per psum group
NGRP = NCH // GRP  # psum groups per batch (4)
SCALE = float(C) ** -0.5


def _bc3(ap2d, reps):
    """[P, W] -> [P, reps, W] broadcast along a new middle dim (step 0)."""
    return bass.AP(
        tensor=ap2d.tensor,
        offset=ap2d.offset,
        ap=[ap2d.ap[0], [0, reps], ap2d.ap[1]],
    )


def _rep3(ap2d, inner):
    """[P, W] -> [P, W, inner] broadcast along a new inner dim (step 0)."""
    return bass.AP(
        tensor=ap2d.tensor,
        offset=ap2d.offset,
        ap=[ap2d.ap[0], ap2d.ap[1], [0, inner]],
    )


def build_program(repeats=1):
    nc = bacc.Bacc(trn_type="TRN2", target_bir_lowering=False, debug=False)

    keyT_d = nc.dram_tensor("keyT", [BPC, 128, KC, N], F32, kind="ExternalInput").ap()
    qT_d = nc.dram_tensor("queryT", [BPC, 128, KC, G], F32, kind="ExternalInput").ap()
    A_d = nc.dram_tensor("Amat", [128, KC, C], F32, kind="ExternalInput").ap()
    u1_d = nc.dram_tensor("u1", [128, KC], F32, kind="ExternalInput").ap()
    wv_d = nc.dram_tensor("WvT", [128, KC, C], F32, kind="ExternalInput").ap()
    wp_d = nc.dram_tensor("WpT", [128, KC, C], F32, kind="ExternalInput").ap()
    pvb_d = nc.dram_tensor("pvb", [1, C], F32, kind="ExternalInput").ap()
    pb_d = nc.dram_tensor("pbias", [1, C], F32, kind="ExternalInput").ap()
    out_d = nc.dram_tensor("out", [BPC, G, C], F32, kind="ExternalOutput").ap()
    soft_d = nc.dram_tensor("soft", [BPC, G, N], F32, kind="ExternalOutput").ap()

    with tile.TileContext(nc) as tc:
        with (
            tc.tile_pool(name="const", bufs=1) as cpool,
            tc.tile_pool(name="kbuf", bufs=2) as kpool,
            tc.tile_pool(name="qbuf", bufs=2) as qpool,
            tc.tile_pool(name="perb", bufs=2) as bpool,
            tc.tile_pool(name="work", bufs=3) as wpool,
            tc.tile_pool(name="stat", bufs=4) as spool,
            tc.tile_pool(name="vsb", bufs=10) as vpool,
            tc.tile_pool(name="sout", bufs=3) as opool,
            tc.tile_pool(name="pat", bufs=2, space="PSUM") as patt,
            tc.tile_pool(name="pvp", bufs=2, space="PSUM") as pvps,
            tc.tile_pool(name="pyv", bufs=1, space="PSUM") as pyv,
            tc.tile_pool(name="ptr", bufs=1, space="PSUM") as ptrp,
            tc.tile_pool(name="pmisc", bufs=2, space="PSUM") as pmisc,
        ):
            # ---- constants ----
            ident = cpool.tile([128, 128], F32)
            make_identity(nc, ident[:])
            A_t = cpool.tile([128, KC, C], F32)
            nc.sync.dma_start(out=A_t[:], in_=A_d)
            u1_t = cpool.tile([128, KC], F32)
            nc.sync.dma_start(out=u1_t[:], in_=u1_d)
            wv_t = cpool.tile([128, KC, C], F32R)
            nc.sync.dma_start(out=wv_t[:], in_=wv_d.bitcast(F32R))
            wp_t = cpool.tile([128, KC, C], F32R)
            nc.sync.dma_start(out=wp_t[:], in_=wp_d.bitcast(F32R))
            pvb_t = cpool.tile([1, C], F32R)
            nc.sync.dma_start(out=pvb_t[:], in_=pvb_d.bitcast(F32R))
            pb_row = cpool.tile([1, C], F32)
            nc.sync.dma_start(out=pb_row[:], in_=pb_d)
            pb_t = cpool.tile([64, C], F32)
            nc.gpsimd.partition_broadcast(pb_t[:], pb_row[:], channels=64)

            for _ in range(repeats):
                for b in range(BPC):
                    kT = kpool.tile([128, KC, N], F32R)
                    nc.sync.dma_start(out=kT[:], in_=keyT_d[b].bitcast(F32R))
                    qT = qpool.tile([128, KC, G], F32)
                    nc.sync.dma_start(out=qT[:], in_=qT_d[b])

                    # tT[c2,g] = sum_c1 A[c1,c2] queryT[c1,g]; c1[g] alongside
                    tcp = pmisc.tile([128, 320], F32, tag="misc")
                    for j in range(KC):
                        for k in range(KC):
                            nc.tensor.matmul(
                                tcp[:, j * 64 : (j + 1) * 64],
                                A_t[:, k, j * 128 : (j + 1) * 128],
                                qT[:, k, :],
                                start=(k == 0),
                                stop=(k == KC - 1),
                            )
                    for k in range(KC):
                        nc.tensor.matmul(
                            tcp[0:1, 192:256],
                            u1_t[:, k : k + 1],
                            qT[:, k, :],
                            start=(k == 0),
                            stop=(k == KC - 1),
                        )
                    tTs = bpool.tile([128, KC, G], F32, tag="tTs")
                    nc.scalar.copy(out=tTs[:], in_=tcp[:, 0:192].rearrange("p (k g) -> p k g", g=64))
                    c1r = bpool.tile([1, G], F32, tag="c1r")
                    nc.scalar.mul(out=c1r[:], in_=tcp[0:1, 192:256], mul=SCALE)
                    c1b = bpool.tile([128, G], F32, tag="c1b")
                    nc.gpsimd.partition_broadcast(c1b[:], c1r[:], channels=128)

                    yvp = pyv.tile([64, 386], F32)

                    for g8 in range(NGRP):
                        ap_ = patt.tile([128, GRP, G], F32)
                        va_list = []
                        for ch in range(GRP):
                            n0 = (g8 * GRP + ch) * 128
                            for k in range(KC):
                                nc.tensor.matmul(
                                    ap_[:, ch, :],
                                    kT[:, k, n0 : n0 + 128].bitcast(F32),
                                    tTs[:, k, :],
                                    start=(k == 0),
                                    stop=(k == KC - 1),
                                )
                            vp = pvps.tile([128, C], F32)
                            for k in range(KC):
                                nc.tensor.matmul(
                                    vp[:],
                                    kT[:, k, n0 : n0 + 128],
                                    wv_t[:, k, :],
                                    start=(k == 0),
                                    stop=(k == KC - 1),
                                )
                            va = vpool.tile([128, C + 2], F32R)
                            nc.scalar.copy(out=va[:, 0:C], in_=vp[:])
                            nc.vector.memset(va[:, C : C + 1].bitcast(F32), 1.0)
                            nc.vector.memset(va[:, C + 1 : C + 2].bitcast(F32), 0.0)
                            va_list.append(va)

                        # scaled logits: sl = s*raw + s*c1  (broadcast c1 over chunks)
                        sl = wpool.tile([128, GRP, G], F32, tag="sl")
                        nc.vector.scalar_tensor_tensor(
                            out=sl[:],
                            in0=ap_[:],
                            scalar=SCALE,
                            in1=_bc3(c1b[:], GRP),
                            op0=mybir.AluOpType.mult,
                            op1=mybir.AluOpType.add,
                        )
                        ex = wpool.tile([128, GRP, G], F32, tag="ex")
                        nc.scalar.activation(ex[:], sl[:], mybir.ActivationFunctionType.Exp)
                        ssum = spool.tile([128, GRP], F32, tag="ssum")
                        nc.vector.tensor_reduce(
                            out=ssum[:], in_=ex[:], axis=mybir.AxisListType.X,
                            op=mybir.AluOpType.add,
                        )
                        rr = spool.tile([128, GRP], F32, tag="rr")
                        nc.vector.reciprocal(out=rr[:], in_=ssum[:])
                        sf = wpool.tile([128, GRP, G], F32, tag="sf")
                        nc.vector.tensor_tensor(
                            out=sf[:], in0=ex[:], in1=_rep3(rr[:], G),
                            op=mybir.AluOpType.mult,
                        )
                        mx = spool.tile([128, GRP], F32, tag="mx")
                        nc.vector.tensor_reduce(
                            out=mx[:], in_=sl[:], axis=mybir.AxisListType.X,
                            op=mybir.AluOpType.max,
                        )
                        yh = wpool.tile([128, GRP, G], F32R, tag="yh")
                        nc.vector.tensor_tensor(
                            out=yh[:], in0=sl[:], in1=_rep3(mx[:], G),
                            op=mybir.AluOpType.is_equal,
                        )
                        for ch in range(GRP):
                            gch = g8 * GRP + ch
                            nc.tensor.matmul(
                                yvp[:],
                                yh[:, ch, :],
                                va_list[ch][:],
                                start=(gch == 0),
                                stop=(gch == NCH - 1),
                            )
                        # soft -> [G, N] layout via PE transposes, 2 half-groups
                        for h in range(2):
                            stp = ptrp.tile([64, 512], F32)
                            for q in range(4):
                                nc.tensor.transpose(
                                    stp[:, q * 128 : (q + 1) * 128],
                                    sf[:, h * 4 + q, :],
                                    ident[:],
                                )
                            sfs = opool.tile([64, 512], F32, tag="sfs")
                            nc.scalar.copy(out=sfs[:], in_=stp[:])
                            n0 = g8 * 1024 + h * 512
                            nc.sync.dma_start(
                                out=soft_d[b, :, n0 : n0 + 512], in_=sfs[:]
                            )

                    # ---- batch tail: renormalize + output projection ----
                    yvs = bpool.tile([64, 386], F32, tag="yvs")
                    nc.scalar.copy(out=yvs[:], in_=yvp[:])
                    cnt1 = spool.tile([64, 1], F32, tag="cnt1")
                    nc.vector.tensor_scalar_add(cnt1[:], yvs[:, 384:385], 1.0)
                    rcol = spool.tile([64, 1], F32, tag="rcol")
                    nc.vector.reciprocal(out=rcol[:], in_=cnt1[:])
                    tvp = pmisc.tile([128, 320], F32, tag="misc")
                    for j in range(KC):
                        nc.tensor.transpose(
                            tvp[0:128, j * 64 : (j + 1) * 64],
                            yvs[:, j * 128 : (j + 1) * 128],
                            ident[0:64, 0:64],
                        )
                    nc.tensor.transpose(
                        tvp[0:1, 192:256], yvs[:, 384:385], ident[0:64, 0:64]
                    )
                    yvt = bpool.tile([128, KC, 64], F32R, tag="yvt")
                    nc.scalar.copy(
                        out=yvt[:], in_=tvp[:, 0:192].rearrange("p (k g) -> p k g", g=64)
                    )
                    cntr = bpool.tile([1, 64], F32R, tag="cntr")
                    nc.scalar.copy(out=cntr[:], in_=tvp[0:1, 192:256])
                    fpt = pmisc.tile([64, 385], F32, tag="misc")
                    fp = fpt[:, 0:384]
                    for j in range(KC):
                        nc.tensor.matmul(
                            fp, yvt[:, j, :], wp_t[:, j, :],
                            start=(j == 0), stop=False,
                        )
                    nc.tensor.matmul(fp, cntr[:], pvb_t[:], start=False, stop=True)
                    ob = opool.tile([64, C], F32, tag="ob")
                    nc.vector.scalar_tensor_tensor(
                        out=ob[:], in0=fp, scalar=rcol[:], in1=pb_t[:],
                        op0=mybir.AluOpType.mult, op1=mybir.AluOpType.add,
                    )
                    nc.sync.dma_start(out=out_d[b], in_=ob[:])

    nc.compile()
    return nc


_prog_cache = {}


def _get_prog(repeats=1):
    if repeats not in _prog_cache:
        _prog_cache[repeats] = build_program(repeats)
    return _prog_cache[repeats]


def host_prep(query, key, q_w, q_b, k_w, k_b, v_w, v_b, p_w, p_b):
    """Host-side weight preprocessing + per-core input shard maps."""
    query = np.asarray(query, np.float32)
    key = np.asarray(key, np.float32)
    A = (q_w.astype(np.float64).T @ k_w.astype(np.float64)).astype(np.float32)
    u1 = (q_w.astype(np.float64).T @ k_b.astype(np.float64)).astype(np.float32)
    pvb = (p_w.astype(np.float64) @ v_b.astype(np.float64)).astype(np.float32)

    A_p = np.ascontiguousarray(A.reshape(KC, 128, C).transpose(1, 0, 2))
    u1_p = np.ascontiguousarray(u1.reshape(KC, 128).T)
    wv_p = np.ascontiguousarray(
        v_w.astype(np.float32).T.reshape(KC, 128, C).transpose(1, 0, 2)
    )
    wp_p = np.ascontiguousarray(
        p_w.astype(np.float32).T.reshape(KC, 128, C).transpose(1, 0, 2)
    )
    pvb_p = pvb.reshape(1, C)
    pb_p = p_b.astype(np.float32).reshape(1, C)

    # [B, X, C] -> [B, 128, KC, X]
    qT = np.ascontiguousarray(query.reshape(B, G, KC, 128).transpose(0, 3, 2, 1))
    kT = np.ascontiguousarray(key.reshape(B, N, KC, 128).transpose(0, 3, 2, 1))

    in_maps = []
    for c in range(NCORES):
        s = slice(c * BPC, (c + 1) * BPC)
        in_maps.append(
            {
                "keyT": kT[s],
                "queryT": qT[s],
                "Amat": A_p,
                "u1": u1_p,
                "WvT": wv_p,
                "WpT": wp_p,
                "pvb": pvb_p,
                "pbias": pb_p,
            }
        )
    return in_maps


def kernel(query, key, q_w, q_b, k_w, k_b, v_w, v_b, p_w, p_b):
    in_maps = host_prep(query, key, q_w, q_b, k_w, k_b, v_w, v_b, p_w, p_b)
    nc = _get_prog(repeats=1)
    res = bass_utils.run_bass_kernel_spmd(nc, in_maps, list(range(NCORES)))
    out = np.concatenate([res.results[c]["out"] for c in range(NCORES)], axis=0)
    soft = np.concatenate([res.results[c]["soft"] for c in range(NCORES)], axis=0)
    return out.astype(np.float32), soft.astype(np.float32)


# revision 8
# speedup vs baseline: 596.6840x; 596.6840x over previous
"""GroupViT AssignAttention (eval branch) on 8 Trainium2 NeuronCores.

Data-parallel over batch: 32 batches -> 4 per core. Each core runs an
identical Bass/Tile program on its own batch shard; no collectives.

Math (per batch, G=64 groups, N=4096 tokens, C=384):
  q = query @ q_w.T + q_b ; k = key @ k_w.T + k_b ; v = key @ v_w.T + v_b
  raw = (q @ k.T) * s,  s = C**-0.5
  soft = softmax_G(raw)                                    [output 2]
  y_hard = onehot(argmax_G(soft))  (forward value of the STE expression)
  attn = y_hard / (y_hard.sum_N + 1)
  out = (attn @ v) @ p_w.T + p_b                           [output 1]

Key algebraic restructuring (exact up to fp rounding):
  raw[g,n] = query@ (q_w.T @ k_w) @ key.T + c1[g] + c2[n] + c3
  Per-token constants c2, c3 shift every logit of a softmax-over-G column
  equally -> dropped. A = q_w.T@k_w and u1 = q_w.T@k_b are precomputed on
  the host from weights only; c1 = query @ u1 on device. This removes the
  full K projection from the device entirely.
  The V path aggregates one-hot-selected v rows via PE matmul with an
  appended ones column producing per-group counts; v bias and p bias are
  folded in afterwards via a rank-1 matmul (cnt x (p_w@v_b)) and a fused
  scalar_tensor_tensor epilogue.

Layouts: everything feature-major ([C,*], C split into 3 chunks of 128
partitions). Attention logits are produced directly transposed,
attnT [n-tokens(partitions), 64 groups(free)], so softmax / argmax /
one-hot are free-dim vector ops and Y@V contracts over token partitions.

Precision: argmax-critical path (tT, attnT) in fp32 PE matmuls; V path
and output projection in fp32r (reduced-mantissa fp32, full PE rate,
~1.5e-4 relative error - far below thresholds for these smooth outputs).
"""

import numpy as np

import concourse.bass as bass
import concourse.bacc as bacc
import concourse.mybir as mybir
import concourse.tile as tile
from concourse import bass_utils
from concourse.masks import make_identity

F32 = mybir.dt.float32
F32R = mybir.dt.float32r

B, G, N, C = 32, 64, 4096, 384
NCORES = 8
BPC = B // NCORES  # batches per core
KC = C // 128  # feature chunks (3)
NCH = N // 128  # token chunks per batch (32)
GRP = 8  # token chunks per psum group
NGRP = NCH // GRP  # psum groups per batch (4)
SCALE = float(C) ** -0.5


def _bc3(ap2d, reps):
    """[P, W] -> [P, reps, W] broadcast along a new middle dim (step 0)."""
    return bass.AP(
        tensor=ap2d.tensor,
        offset=ap2d.offset,
        ap=[ap2d.ap[0], [0, reps], ap2d.ap[1]],
    )


def _rep3(ap2d, inner):
    """[P, W] -> [P, W, inner] broadcast along a new inner dim (step 0)."""
    return bass.AP(
        tensor=ap2d.tensor,
        offset=ap2d.offset,
        ap=[ap2d.ap[0], ap2d.ap[1], [0, inner]],
    )


def build_program(repeats=1):
    nc = bacc.Bacc(trn_type="TRN2", target_bir_lowering=False, debug=False)

    keyT_d = nc.dram_tensor("keyT", [BPC, 128, KC, N], F32, kind="ExternalInput").ap()
    tT_d = nc.dram_tensor("tT", [BPC, 128, KC, G], F32, kind="ExternalInput").ap()
    c1_d = nc.dram_tensor("c1s", [BPC, 1, G], F32, kind="ExternalInput").ap()
    wv_d = nc.dram_tensor("WvT", [128, KC, C], F32, kind="ExternalInput").ap()
    wp_d = nc.dram_tensor("WpT", [128, KC, C], F32, kind="ExternalInput").ap()
    pvb_d = nc.dram_tensor("pvb", [1, C], F32, kind="ExternalInput").ap()
    pb_d = nc.dram_tensor("pbias", [1, C], F32, kind="ExternalInput").ap()
    out_d = nc.dram_tensor("out", [BPC, G, C], F32, kind="ExternalOutput").ap()
    soft_d = nc.dram_tensor("soft", [BPC, G, N], F32, kind="ExternalOutput").ap()

    with tile.TileContext(nc) as tc:
        with (
            tc.tile_pool(name="const", bufs=1) as cpool,
            tc.tile_pool(name="kbuf", bufs=2) as kpool,
            tc.tile_pool(name="perb", bufs=2) as bpool,
            tc.tile_pool(name="work", bufs=3) as wpool,
            tc.tile_pool(name="stat", bufs=4) as spool,
            tc.tile_pool(name="vsb", bufs=10) as vpool,
            tc.tile_pool(name="sout", bufs=3) as opool,
            tc.tile_pool(name="pat", bufs=2, space="PSUM") as patt,
            tc.tile_pool(name="pvp", bufs=2, space="PSUM") as pvps,
            tc.tile_pool(name="pyv", bufs=1, space="PSUM") as pyv,
            tc.tile_pool(name="ptr", bufs=1, space="PSUM") as ptrp,
            tc.tile_pool(name="pmisc", bufs=2, space="PSUM") as pmisc,
        ):
            # ---- constants ----
            ident = cpool.tile([128, 128], F32)
            make_identity(nc, ident[:])
            wv_t = cpool.tile([128, KC, C], F32R)
            nc.sync.dma_start(out=wv_t[:], in_=wv_d.bitcast(F32R))
            wp_t = cpool.tile([128, KC, C], F32R)
            nc.sync.dma_start(out=wp_t[:], in_=wp_d.bitcast(F32R))
            pvb_t = cpool.tile([1, C], F32R)
            nc.sync.dma_start(out=pvb_t[:], in_=pvb_d.bitcast(F32R))
            pb_row = cpool.tile([1, C], F32)
            nc.sync.dma_start(out=pb_row[:], in_=pb_d)
            pb_t = cpool.tile([64, C], F32)
            nc.gpsimd.partition_broadcast(pb_t[:], pb_row[:], channels=64)

            for _ in range(repeats):
                for b in range(BPC):
                    kT = kpool.tile([128, KC, N], F32R)
                    nc.sync.dma_start(out=kT[:], in_=keyT_d[b].bitcast(F32R))
                    tTs = bpool.tile([128, KC, G], F32, tag="tTs")
                    nc.sync.dma_start(out=tTs[:], in_=tT_d[b])
                    c1r = bpool.tile([1, G], F32, tag="c1r")
                    nc.sync.dma_start(out=c1r[:], in_=c1_d[b])
                    c1b = bpool.tile([128, G], F32, tag="c1b")
                    nc.gpsimd.partition_broadcast(c1b[:], c1r[:], channels=128)

                    yvp = pyv.tile([64, 386], F32)

                    for g8 in range(NGRP):
                        ap_ = patt.tile([128, GRP, G], F32)
                        va_list = []
                        for ch in range(GRP):
                            n0 = (g8 * GRP + ch) * 128
                            for k in range(KC):
                                nc.tensor.matmul(
                                    ap_[:, ch, :],
                                    kT[:, k, n0 : n0 + 128].bitcast(F32),
                                    tTs[:, k, :],
                                    start=(k == 0),
                                    stop=(k == KC - 1),
                                )
                            vp = pvps.tile([128, C], F32)
                            for k in range(KC):
                                nc.tensor.matmul(
                                    vp[:],
                                    kT[:, k, n0 : n0 + 128],
                                    wv_t[:, k, :],
                                    start=(k == 0),
                                    stop=(k == KC - 1),
                                )
                            va = vpool.tile([128, C + 2], F32R)
                            nc.scalar.copy(out=va[:, 0:C], in_=vp[:])
                            nc.vector.memset(va[:, C : C + 1].bitcast(F32), 1.0)
                            nc.vector.memset(va[:, C + 1 : C + 2].bitcast(F32), 0.0)
                            va_list.append(va)

                        # scaled logits: sl = s*raw + s*c1  (broadcast c1 over chunks)
                        sl = wpool.tile([128, GRP, G], F32, tag="sl")
                        nc.vector.scalar_tensor_tensor(
                            out=sl[:],
                            in0=ap_[:],
                            scalar=SCALE,
                            in1=_bc3(c1b[:], GRP),
                            op0=mybir.AluOpType.mult,
                            op1=mybir.AluOpType.add,
                        )
                        ex = wpool.tile([128, GRP, G], F32, tag="ex")
                        nc.scalar.activation(ex[:], sl[:], mybir.ActivationFunctionType.Exp)
                        ssum = spool.tile([128, GRP], F32, tag="ssum")
                        nc.vector.tensor_reduce(
                            out=ssum[:], in_=ex[:], axis=mybir.AxisListType.X,
                            op=mybir.AluOpType.add,
                        )
                        rr = spool.tile([128, GRP], F32, tag="rr")
                        nc.vector.reciprocal(out=rr[:], in_=ssum[:])
                        sf = wpool.tile([128, GRP, G], F32, tag="sf")
                        nc.vector.tensor_tensor(
                            out=sf[:], in0=ex[:], in1=_rep3(rr[:], G),
                            op=mybir.AluOpType.mult,
                        )
                        mx = spool.tile([128, GRP], F32, tag="mx")
                        nc.vector.tensor_reduce(
                            out=mx[:], in_=sl[:], axis=mybir.AxisListType.X,
                            op=mybir.AluOpType.max,
                        )
                        yh = wpool.tile([128, GRP, G], F32R, tag="yh")
                        nc.vector.tensor_tensor(
                            out=yh[:], in0=sl[:], in1=_rep3(mx[:], G),
                            op=mybir.AluOpType.is_equal,
                        )
                        for ch in range(GRP):
                            gch = g8 * GRP + ch
                            nc.tensor.matmul(
                                yvp[:],
                                yh[:, ch, :],
                                va_list[ch][:],
                                start=(gch == 0),
                                stop=(gch == NCH - 1),
                            )
                        # soft -> [G, N] layout via PE transposes, 2 half-groups
                        for h in range(2):
                            stp = ptrp.tile([64, 512], F32)
                            for q in range(4):
                                nc.tensor.transpose(
                                    stp[:, q * 128 : (q + 1) * 128],
                                    sf[:, h * 4 + q, :],
                                    ident[:],
                                )
                            sfs = opool.tile([64, 512], F32, tag="sfs")
                            nc.scalar.copy(out=sfs[:], in_=stp[:])
                            n0 = g8 * 1024 + h * 512
                            nc.sync.dma_start(
                                out=soft_d[b, :, n0 : n0 + 512], in_=sfs[:]
                            )

                    # ---- batch tail: renormalize + output projection ----
                    yvs = bpool.tile([64, 386], F32, tag="yvs")
                    nc.scalar.copy(out=yvs[:], in_=yvp[:])
                    cnt1 = spool.tile([64, 1], F32, tag="cnt1")
                    nc.vector.tensor_scalar_add(cnt1[:], yvs[:, 384:385], 1.0)
                    rcol = spool.tile([64, 1], F32, tag="rcol")
                    nc.vector.reciprocal(out=rcol[:], in_=cnt1[:])
                    tvp = pmisc.tile([128, 320], F32, tag="misc")
                    for j in range(KC):
                        nc.tensor.transpose(
                            tvp[0:128, j * 64 : (j + 1) * 64],
                            yvs[:, j * 128 : (j + 1) * 128],
                            ident[0:64, 0:64],
                        )
                    nc.tensor.transpose(
                        tvp[0:1, 192:256], yvs[:, 384:385], ident[0:64, 0:64]
                    )
                    yvt = bpool.tile([128, KC, 64], F32R, tag="yvt")
                    nc.scalar.copy(
                        out=yvt[:], in_=tvp[:, 0:192].rearrange("p (k g) -> p k g", g=64)
                    )
                    cntr = bpool.tile([1, 64], F32R, tag="cntr")
                    nc.scalar.copy(out=cntr[:], in_=tvp[0:1, 192:256])
                    fpt = pmisc.tile([64, 385], F32, tag="misc")
                    fp = fpt[:, 0:384]
                    for j in range(KC):
                        nc.tensor.matmul(
                            fp, yvt[:, j, :], wp_t[:, j, :],
                            start=(j == 0), stop=False,
                        )
                    nc.tensor.matmul(fp, cntr[:], pvb_t[:], start=False, stop=True)
                    ob = opool.tile([64, C], F32, tag="ob")
                    nc.vector.scalar_tensor_tensor(
                        out=ob[:], in0=fp, scalar=rcol[:], in1=pb_t[:],
                        op0=mybir.AluOpType.mult, op1=mybir.AluOpType.add,
                    )
                    nc.sync.dma_start(out=out_d[b], in_=ob[:])

    nc.compile()
    return nc


_prog_cache = {}


def _get_prog(repeats=1):
    if repeats not in _prog_cache:
        _prog_cache[repeats] = build_program(repeats)
    return _prog_cache[repeats]


def host_prep(query, key, q_w, q_b, k_w, k_b, v_w, v_b, p_w, p_b):
    """Host-side weight preprocessing + per-core input shard maps."""
    query = np.asarray(query, np.float32)
    key = np.asarray(key, np.float32)
    A = q_w.astype(np.float64).T @ k_w.astype(np.float64)
    u1 = q_w.astype(np.float64).T @ k_b.astype(np.float64)
    pvb = (p_w.astype(np.float64) @ v_b.astype(np.float64)).astype(np.float32)

    # t = query @ A and c1 = query @ u1 in fp64 (tiny: 1.3% of total FLOPs).
    # Device-side fp32 PE rounding of t would amplify x sqrt(C) through the
    # attention matmul and flip near-tie argmaxes vs the reference.
    tmat = np.einsum("bgc,cd->bgd", query.astype(np.float64), A)
    c1 = SCALE * (query.astype(np.float64) @ u1)  # [B, G]
    tT_p = np.ascontiguousarray(
        tmat.transpose(0, 2, 1).reshape(B, KC, 128, G).transpose(0, 2, 1, 3)
    ).astype(np.float32)
    c1_p = c1.astype(np.float32).reshape(B, 1, G)
    wv_p = np.ascontiguousarray(
        v_w.astype(np.float32).T.reshape(KC, 128, C).transpose(1, 0, 2)
    )
    wp_p = np.ascontiguousarray(
        p_w.astype(np.float32).T.reshape(KC, 128, C).transpose(1, 0, 2)
    )
    pvb_p = pvb.reshape(1, C)
    pb_p = p_b.astype(np.float32).reshape(1, C)

    # [B, N, C] -> [B, 128, KC, N]
    kT = np.ascontiguousarray(key.reshape(B, N, KC, 128).transpose(0, 3, 2, 1))

    in_maps = []
    for c in range(NCORES):
        s = slice(c * BPC, (c + 1) * BPC)
        in_maps.append(
            {
                "keyT": kT[s],
                "tT": tT_p[s],
                "c1s": c1_p[s],
                "WvT": wv_p,
                "WpT": wp_p,
                "pvb": pvb_p,
                "pbias": pb_p,
            }
        )
    return in_maps


def kernel(query, key, q_w, q_b, k_w, k_b, v_w, v_b, p_w, p_b):
    in_maps = host_prep(query, key, q_w, q_b, k_w, k_b, v_w, v_b, p_w, p_b)
    nc = _get_prog(repeats=1)
    res = bass_utils.run_bass_kernel_spmd(nc, in_maps, list(range(NCORES)))
    out = np.concatenate([res.results[c]["out"] for c in range(NCORES)], axis=0)
    soft = np.concatenate([res.results[c]["soft"] for c in range(NCORES)], axis=0)
    return out.astype(np.float32), soft.astype(np.float32)
